# revision 1
# baseline (speedup 1.0000x reference)
"""Multi-head SAGE layer (mean aggregation) as a Bass/Tile kernel on 8 trn2 cores.

Math: out = mean_h( h @ W_self[h] + segmean(h[src] by dst) @ W_neigh[h] + b[h] )
    = h @ mean_h(W_self) + segmean(h[src] by dst) @ mean_h(W_neigh) + mean_h(b)
(mean over heads commutes with the linear layers).

Sharding: nodes (and their incident edges, keyed by dst) are split across the
8 cores, 12500 nodes each; h is replicated per-core as the gather table.
Per core, dst nodes are processed in blocks of 128.  For each block, the
edges' source rows are fetched with dma_gather (the GPSIMD Ant gather):
since its indices are int16, the table is covered by 4 source-range buckets
of 25000 rows, and each block issues up to 4 gather calls, one per bucket,
pinned to the 4 SWDGE queues so descriptor generation runs 4-way parallel
(the descriptor rate is the kernel's bottleneck).  The segment-sum runs on
the tensor engine: per 128-edge sub-tile a 0/1 selection matrix
M[e, n] = (dst_local[e] == n) is built with a vector is_equal against an
iota row, and M.T @ gathered accumulates [node, feat] in PSUM.  Degree
normalization is a per-partition scalar multiply; a PE transpose feeds the
two output matmuls (neigh and self terms) plus the head-averaged bias.

All graph-structure preprocessing (edge partition/sort/padding, degree
counts, layout transposes) happens on the host; all floating-point math on
the features/weights happens on-device.
"""

import sys

import numpy as np

for _p in ("/opt/trn_rl_repo",):
    if _p not in sys.path:
        sys.path.insert(0, _p)

N_NODES = 100000
N_EDGES = 1600000
D = 128
H = 4
N_CORES = 8
P = 128
NPC = N_NODES // N_CORES          # nodes per core
NB = (NPC + P - 1) // P           # 128-node blocks per core
NPAD = NB * P                     # padded nodes per core
NBUCKETS = 4
BUCKET_SZ = -(-N_NODES // NBUCKETS)   # src rows per gather bucket (int16 range)
MAX_CALL_TILES = 8                # dma_gather accepts at most 1024 indices


def _preprocess(src, dst):
    """Partition edges by dst owner core, group by 128-node dst block and
    src bucket, sort by src within each group, and pad each (block, bucket)
    group's edge list to R[b,k] = T[b,k]*128 gather slots (T = max count
    over the 8 cores, ceil-divided by 128, so the SPMD schedule is shared;
    pad slots gather row 0 of the bucket and are masked out of the
    segment-sum by rseg = -1).

    Returns (per_core, R, T, col_off, w_off, sumT, sumW) where per_core[c]:
      idx16 [128, sumW] int16   bucket-local gather indices, dma_gather
                                wrapped-16 layout, replicated across the
                                eight 16-partition groups; pad = 0
      rseg  [128, sumT] float32 dst id local to the block in [0,128),
                                pad = -1; column order (b, k, t)
      deg   [128, NB]   int32   in-degree per node, column b = block b
    Slot j of group (b, k) is edge sub-tile t = j // 128, partition j % 128.
    """
    deg_full = np.bincount(dst, minlength=N_NODES).astype(np.int32)
    counts = np.zeros((N_CORES, NB, NBUCKETS), np.int64)
    blocks = []
    for c in range(N_CORES):
        lo = c * NPC
        m = (dst >= lo) & (dst < lo + NPC)
        s_c = src[m].astype(np.int64)
        d_c = (dst[m] - lo).astype(np.int64)
        blk = d_c >> 7
        bkt = s_c // BUCKET_SZ
        order = np.lexsort((s_c, bkt, blk))
        s_c, d_c, blk, bkt = s_c[order], d_c[order], blk[order], bkt[order]
        key = (blk * NBUCKETS + bkt)
        bc = np.bincount(key, minlength=NB * NBUCKETS)
        counts[c] = bc.reshape(NB, NBUCKETS)
        off = np.zeros(NB * NBUCKETS + 1, np.int64)
        np.cumsum(bc, out=off[1:])
        blocks.append((s_c, d_c, off))

    cmax = counts.max(axis=0)                       # [NB, NBUCKETS]
    T = (-(-np.maximum(1, cmax) // P)).astype(np.int64)  # matmul sub-tiles
    R = T * P                                       # gather slots
    # column offsets: sub-tiles (rseg/msel) and int16 idx columns, (b, k)
    col_off = np.zeros((NB, NBUCKETS), np.int64)
    w_off = np.zeros((NB, NBUCKETS), np.int64)
    acc_t = 0
    acc_w = 0
    for b in range(NB):
        for k in range(NBUCKETS):
            col_off[b, k] = acc_t
            w_off[b, k] = acc_w
            acc_t += T[b, k]
            acc_w += -(-R[b, k] // 16)
    sumT = int(acc_t)
    sumW = int(acc_w)

    per_core = []
    for c in range(N_CORES):
        s_c, d_c, off = blocks[c]
        idx_cols = np.zeros((16, sumW), np.int16)
        rseg_flat = np.full(sumT * P, -1.0, np.float32)
        for b in range(NB):
            for k in range(NBUCKETS):
                n = int(counts[c, b, k])
                w = int(-(-R[b, k] // 16))
                if n > 0:
                    o = int(off[b * NBUCKETS + k])
                    flat = np.zeros(w * 16, np.int64)
                    flat[:n] = s_c[o:o + n] - k * BUCKET_SZ
                    w0 = int(w_off[b, k])
                    idx_cols[:, w0:w0 + w] = flat.reshape(w, 16).T
                    base = int(col_off[b, k]) * P
                    rseg_flat[base:base + n] = (d_c[o:o + n] - b * P).astype(
                        np.float32)
        idx16 = np.tile(np.ascontiguousarray(idx_cols), (8, 1))
        rseg_t = np.ascontiguousarray(rseg_flat.reshape(sumT, P).T)
        degc = np.zeros(NPAD, np.int32)
        degc[:NPC] = deg_full[c * NPC:(c + 1) * NPC]
        deg_t = np.ascontiguousarray(degc.reshape(NB, P).T)
        per_core.append({"idx16": idx16, "rseg": rseg_t, "deg": deg_t})
    return per_core, R, T, col_off, w_off, sumT, sumW


def build_program(R, T, col_off, w_off, sumT, sumW,
                  n_nodes=N_NODES, nb=NB, npad=NPAD):
    """Trace + compile the SPMD Bass program for the given group schedule."""
    from contextlib import ExitStack

    from concourse import bacc, mybir, tile
    from concourse.masks import make_identity

    f32 = mybir.dt.float32
    bf16 = mybir.dt.bfloat16
    i32 = mybir.dt.int32
    i16 = mybir.dt.int16
    AL = mybir.AluOpType

    nc = bacc.Bacc("TRN2", target_bir_lowering=False, debug=False,
                   num_devices=N_CORES, num_swdge_queues=NBUCKETS)
    h_ap = nc.dram_tensor("h_table", [n_nodes, D], f32, kind="ExternalInput").ap()
    hT_ap = nc.dram_tensor("hT", [P, npad], f32, kind="ExternalInput").ap()
    idx_ap = nc.dram_tensor("idx16", [P, sumW], i16, kind="ExternalInput").ap()
    rseg_ap = nc.dram_tensor("rseg", [P, sumT], f32, kind="ExternalInput").ap()
    deg_ap = nc.dram_tensor("deg", [P, nb], i32, kind="ExternalInput").ap()
    iota_ap = nc.dram_tensor("iota", [P, P], f32, kind="ExternalInput").ap()
    ws_ap = nc.dram_tensor("W_self", [H, D, D], f32, kind="ExternalInput").ap()
    wn_ap = nc.dram_tensor("W_neigh", [H, D, D], f32, kind="ExternalInput").ap()
    b_ap = nc.dram_tensor("b", [H, D], f32, kind="ExternalInput").ap()
    out_ap = nc.dram_tensor("out", [npad, D], f32, kind="ExternalOutput").ap()

    bucket_aps = []
    for k in range(NBUCKETS):
        lo = k * BUCKET_SZ
        hi = min(n_nodes, lo + BUCKET_SZ)
        bucket_aps.append(h_ap[lo:hi, :])

    with tile.TileContext(nc) as tc, ExitStack() as ctx:
        const = ctx.enter_context(tc.tile_pool(name="const", bufs=1))
        eps = [ctx.enter_context(tc.tile_pool(name=f"eg{k}", bufs=3))
               for k in range(NBUCKETS)]
        mp = ctx.enter_context(tc.tile_pool(name="msel", bufs=4))
        sp = ctx.enter_context(tc.tile_pool(name="small", bufs=3))
        pseg = ctx.enter_context(tc.tile_pool(name="pseg", bufs=2, space="PSUM"))
        ptr = ctx.enter_context(tc.tile_pool(name="ptr", bufs=2, space="PSUM"))
        pout = ctx.enter_context(tc.tile_pool(name="pout", bufs=2, space="PSUM"))
        ppro = ctx.enter_context(tc.tile_pool(name="ppro", bufs=1, space="PSUM"))

        # ---- prologue: constants ----
        iota = const.tile([P, P], f32, tag="iota")
        nc.sync.dma_start(iota[:], iota_ap)
        ident = const.tile([P, P], f32, tag="ident")
        make_identity(nc, ident[:])

        # head-averaged weights: wm = 0.25 * sum_h W[h]
        wmeans = []
        for name, ap in (("ws", ws_ap), ("wn", wn_ap)):
            heads = []
            for hh in range(H):
                t = const.tile([P, P], f32, tag=f"{name}h{hh}")
                nc.sync.dma_start(t[:], ap[hh])
                heads.append(t)
            s01 = const.tile([P, P], f32, tag=f"{name}s01")
            nc.vector.tensor_tensor(s01[:], heads[0][:], heads[1][:], op=AL.add)
            s23 = const.tile([P, P], f32, tag=f"{name}s23")
            nc.vector.tensor_tensor(s23[:], heads[2][:], heads[3][:], op=AL.add)
            s = const.tile([P, P], f32, tag=f"{name}sum")
            nc.vector.tensor_tensor(s[:], s01[:], s23[:], op=AL.add)
            wm = const.tile([P, P], f32, tag=f"{name}m")
            nc.scalar.mul(wm[:], s[:], 1.0 / H)
            wmeans.append(wm)
        wsm, wnm = wmeans

        # head-averaged bias replicated across partitions:
        # lhsT = (1/H)-filled [H, P]  ->  out[p, o] = (1/H) * sum_h b[h, o]
        b_sb = const.tile([H, P], f32, tag="bsb")
        nc.sync.dma_start(b_sb[:], b_ap)
        q = const.tile([H, P], f32, tag="q")
        nc.vector.memset(q[:], 1.0 / H)
        pb = ppro.tile([P, P], f32, tag="pb")
        nc.tensor.matmul(pb[:], lhsT=q[:], rhs=b_sb[:], start=True, stop=True)
        bias = const.tile([P, P], f32, tag="bias")
        nc.vector.tensor_copy(bias[:], pb[:])

        # inverse degree: 1 / max(deg, 1)
        degsb = const.tile([P, nb], i32, tag="degsb")
        nc.sync.dma_start(degsb[:], deg_ap)
        degf = const.tile([P, nb], f32, tag="degf")
        nc.vector.tensor_copy(degf[:], degsb[:])
        nc.vector.tensor_scalar_max(degf[:], degf[:], 1.0)
        invd = const.tile([P, nb], f32, tag="invd")
        nc.vector.reciprocal(invd[:], degf[:])

        # edge structure, resident in SBUF
        idx_all = const.tile([P, sumW], i16, tag="idx_all")
        nc.sync.dma_start(idx_all[:], idx_ap)
        rseg_all = const.tile([P, sumT], f32, tag="rseg_all")
        nc.sync.dma_start(rseg_all[:], rseg_ap)

        tmax = [max(int(T[b, k]) for b in range(nb)) for k in range(NBUCKETS)]

        # ---- main loop over 128-node dst blocks ----
        for b in range(nb):
            etiles = {}
            for k in range(NBUCKETS):
                Rk = int(R[b, k])
                if Rk == 0:
                    continue
                E = eps[k].tile([P, tmax[k] * P], f32, tag=f"E{k}")
                for j0 in range(0, Rk, 1024):
                    n = min(1024, Rk - j0)
                    ntile = -(-n // P)
                    w0 = int(w_off[b, k]) + j0 // 16
                    nc.gpsimd.dma_gather(
                        E[:, j0:j0 + ntile * P].rearrange(
                            "p (c d) -> p c d", d=D),
                        bucket_aps[k],
                        idx_all[:, w0:w0 + (-(-n // 16))],
                        n,
                        n,
                        D,
                        queue_num=k,
                    )
                etiles[k] = E

            ps = pseg.tile([P, P], f32, tag="seg")
            nmm = int(T[b].sum())
            i = 0
            for k in range(NBUCKETS):
                Tk = int(T[b, k])
                if Tk == 0:
                    continue
                c0 = int(col_off[b, k])
                for t in range(Tk):
                    msel = mp.tile([P, P], f32, tag="msel")
                    nc.vector.tensor_tensor(
                        out=msel[:],
                        in0=rseg_all[:, c0 + t:c0 + t + 1].to_broadcast([P, P]),
                        in1=iota[:],
                        op=AL.is_equal,
                    )
                    nc.tensor.matmul(
                        ps[:],
                        lhsT=msel[:],
                        rhs=etiles[k][:, t * P:(t + 1) * P],
                        start=(i == 0),
                        stop=(i == nmm - 1),
                    )
                    i += 1

            # h_neigh block [node, feat], degree-normalized
            hngh = sp.tile([P, P], f32, tag="hngh")
            nc.vector.tensor_scalar(out=hngh[:], in0=ps[:],
                                    scalar1=invd[:, b:b + 1], scalar2=None,
                                    op0=AL.mult)
            psT = ptr.tile([P, P], f32, tag="T")
            nc.tensor.transpose(psT[:], hngh[:], ident[:])
            hnghT = sp.tile([P, P], f32, tag="hnghT")
            nc.scalar.copy(hnghT[:], psT[:])
            hTt = sp.tile([P, P], f32, tag="hTt")
            nc.sync.dma_start(hTt[:], hT_ap[:, b * P:(b + 1) * P])
            po = pout.tile([P, P], f32, tag="out")
            nc.tensor.matmul(po[:], lhsT=hnghT[:], rhs=wnm[:],
                             start=True, stop=False)
            nc.tensor.matmul(po[:], lhsT=hTt[:], rhs=wsm[:],
                             start=False, stop=True)
            ob = sp.tile([P, P], f32, tag="ob")
            nc.vector.tensor_tensor(out=ob[:], in0=po[:], in1=bias[:], op=AL.add)
            nc.sync.dma_start(out_ap[b * P:(b + 1) * P, :], ob[:])

    nc.compile()
    return nc


_CACHE = {}


def kernel(h, src, dst, W_self, W_neigh, b):
    return run(h, src, dst, W_self, W_neigh, b)[0]


def run(h, src, dst, W_self, W_neigh, b, trace=False, **kw):
    from concourse.bass_utils import run_bass_kernel_spmd

    h = np.ascontiguousarray(np.asarray(h, dtype=np.float32))
    src = np.asarray(src, dtype=np.int32)
    dst = np.asarray(dst, dtype=np.int32)
    W_self = np.ascontiguousarray(np.asarray(W_self, dtype=np.float32))
    W_neigh = np.ascontiguousarray(np.asarray(W_neigh, dtype=np.float32))
    b = np.ascontiguousarray(np.asarray(b, dtype=np.float32))

    per_core, R, T, col_off, w_off, sumT, sumW = _preprocess(src, dst)

    key = (tuple(R.ravel().tolist()),)
    if key not in _CACHE:
        _CACHE[key] = build_program(R, T, col_off, w_off, sumT, sumW)
    nc = _CACHE[key]

    iota = np.ascontiguousarray(
        np.tile(np.arange(P, dtype=np.float32), (P, 1)))
    in_maps = []
    for c in range(N_CORES):
        hTc = np.zeros((P, NPAD), np.float32)
        hTc[:, :NPC] = h[c * NPC:(c + 1) * NPC].T
        in_maps.append({
            "h_table": h,
            "hT": np.ascontiguousarray(hTc),
            "idx16": per_core[c]["idx16"],
            "rseg": per_core[c]["rseg"],
            "deg": per_core[c]["deg"],
            "iota": iota,
            "W_self": W_self,
            "W_neigh": W_neigh,
            "b": b,
        })

    res = run_bass_kernel_spmd(nc, in_maps, core_ids=list(range(N_CORES)),
                               trace=trace, **kw)
    out = np.concatenate([res.results[c]["out"][:NPC] for c in range(N_CORES)],
                         axis=0)
    return out, res

